# revision 35
# baseline (speedup 1.0000x reference)
"""Trainium2 Bass kernel for nn_Attentionv2 (B=8, N=1024, C=768, H=12, D=64).

Strategy: data-parallel over batch — one batch element per NeuronCore (8 cores).
Per core, multi-head attention is computed entirely in the "transposed"
orientation so no on-chip transposes are needed:

  QT[h*64+d, n] = sum_c WqT[c, h*64+d] * xT[c, n]     (head-pair tiles)
  KT likewise; V[n, h*64+d] = sum_c xT[c, n-tile] * WvT[c, :]
  ST[m, n]  = sum_d KT[d, m] * QT[d, n]               (scores transposed)
  ET        = exp(ST * 1/8)                            (no max-subtraction:
                                                        scores are O(1) here)
  PV lhsT   = [V_h | ones(64 cols)]  =>  out rows 0-63 = OT_h (unnorm),
               rows 64-127 = softmax denominator replicated 64x
  OT_norm   = OT * (1/Z)                               (reciprocal via DVE)
  y[n, o]   = sum_c OT_norm[c, n] * WpT[c, o] + bp[o]

All matmul operands are fp16: fp8 was tried and rejected — the attention
output is a near-uniform mean over 1024 keys, so quantization noise on
V/ET/scores passes through at full relative strength (~5% >> the 2e-2
gate).  Accumulation is fp32 in PSUM.

Schedule: host-prepped chunk-contiguous input layouts; pair-0 weights
land first and QK(0) accumulates as x arrives so the softmax pipe starts
early; dummy warm-up matmuls spin the PE from t~6us so HAM unthrottles
to 2.4GHz before real work; V-projection waves run under the pair-0/1
softmax windows; scores PSUM is triple-buffered so pair-boundary WARs
don't stall the ACT pipe; output projection interleaves with the last
pair's PV; y streams out over three DMA queues.
"""

import numpy as np

P = 128
B, N, C = 8, 1024, 768
H, D = 12, 64
SCALE = D ** -0.5  # 0.125
CT = C // P   # 6 contraction chunks
NT = N // P   # 8 sequence tiles
HP = H // 2   # 6 head pairs
NCORES = 8
NWARM = 12    # dummy PE warm-up matmuls

_cache = {}


def _build_nc():
    import concourse.bass as bass
    import concourse.mybir as mybir
    import concourse.tile as tile
    from concourse import bacc

    f32 = mybir.dt.float32
    f16 = mybir.dt.float16
    Exp = mybir.ActivationFunctionType.Exp

    nc = bacc.Bacc("TRN2", target_bir_lowering=False, debug=False,
                   enable_asserts=False)

    # host-prepped layouts (see _make_in_maps):
    #   xc  [CT, 128, N]        f16   x^T c-chunked
    #   wq  [HP, 128, CT, 128]  f16   per head-pair j: [p, c, (hh d)]
    #   wk  [HP, 128, CT, 128]  f16
    #   wv  [CT, 128, H*D]      f16
    #   wp  [CT, 128, C]        f16
    #   bpb [128, C]            f32   bias broadcast
    xc = nc.dram_tensor("xc", [CT, P, N], f16, kind="ExternalInput").ap()
    wq = nc.dram_tensor("wq", [HP, P, CT, P], f16, kind="ExternalInput").ap()
    wk = nc.dram_tensor("wk", [HP, P, CT, P], f16, kind="ExternalInput").ap()
    wv = nc.dram_tensor("wv", [CT, P, H * D], f16, kind="ExternalInput").ap()
    wp = nc.dram_tensor("wp", [CT, P, C], f16, kind="ExternalInput").ap()
    bpb = nc.dram_tensor("bpb", [P, C], f32, kind="ExternalInput").ap()
    y = nc.dram_tensor("y", [N, C], f32, kind="ExternalOutput").ap()

    mm = nc.tensor.matmul

    with tile.TileContext(nc) as tc:
        with tc.tile_pool(name="persist", bufs=1) as persist, \
             tc.tile_pool(name="mix", bufs=2, space="PSUM") as mix, \
             tc.tile_pool(name="ps_o", bufs=2, space="PSUM") as ps_o, \
             tc.tile_pool(name="et", bufs=32) as etp, \
             tc.tile_pool(name="sm", bufs=4) as smp, \
             tc.tile_pool(name="ps_s", bufs=2, space="PSUM") as ps_s, \
             tc.tile_pool(name="outp", bufs=3) as outp:
            qt = persist.tile([P, HP, N], f16)        # QT: head pair j rows
            kt = persist.tile([P, HP, N], f16)
            vp = persist.tile([P, NT, H, 2 * D], f16)  # [Vh | ones]
            ot = persist.tile([P, HP, N], f16)        # normalized OT stacked
            x_sb = persist.tile([P, CT, N], f16)
            wq_sb = persist.tile([P, HP, CT, P], f16)
            wk_sb = persist.tile([P, HP, CT, P], f16)
            wv_sb = persist.tile([P, CT, H * D], f16)
            wp_sb = persist.tile([P, CT, C], f16)
            bpb_sb = persist.tile([P, C], f32)
            warm = persist.tile([P, 512], f16)

            nc.vector.memset(warm[:], 0.125)

            # ---- input DMA: chunk-contiguous; queue FIFO acts as a
            # throttle — late-needed weights sit behind x/wv on the HWDGE
            # queues so they can't steal bandwidth from the critical path;
            # gpsimd carries only the early pair-0/1 weights ----
            # scalar queue carries ONLY the three early x chunks: its
            # sequencer must be free before the first EXP dispatches.
            # All late-needed weights go on the self-throttled gpsimd
            # queue (dedicated engine, ~1 transfer / 1.5us — plenty).
            xq = [nc.sync, nc.scalar]
            for c in range(CT):
                xq[c % 2].dma_start(x_sb[:, c, :], xc[c])
            nc.gpsimd.dma_start(wq_sb[:, 0], wq[0])
            nc.gpsimd.dma_start(wk_sb[:, 0], wk[0])
            nc.gpsimd.dma_start(wq_sb[:, 1], wq[1])
            nc.gpsimd.dma_start(wk_sb[:, 1], wk[1])
            nc.sync.dma_start(wv_sb[:, 0:3],
                              wv[0:3].rearrange("c p f -> p c f"))
            nc.gpsimd.dma_start(wv_sb[:, 3:6],
                                wv[3:6].rearrange("c p f -> p c f"))
            for j in range(2, HP):
                nc.gpsimd.dma_start(wq_sb[:, j], wq[j])
                nc.gpsimd.dma_start(wk_sb[:, j], wk[j])
            nc.gpsimd.dma_start(wp_sb[:], wp.rearrange("c p f -> p c f"))
            nc.gpsimd.dma_start(bpb_sb[:], bpb)

            # ---- PE warm-up: dummy matmuls so HAM unthrottles to 2.4GHz
            # during the DMA window ----
            wps = ps_o.tile([P, 512], f32, tag="o", name="warmps")
            for i in range(NWARM):
                mm(wps[:], lhsT=warm[:, 0:128], rhs=warm[:],
                   start=(i == 0), stop=(i == NWARM - 1))
            nc.vector.tensor_copy(warm[:], wps[:])  # keep it live (no DCE)

            # ---- QK projection for pair j ----
            def emit_qk(j, half=None):
                pairs = ((wq_sb, qt), (wk_sb, kt))
                if half is not None:
                    pairs = (pairs[half],)
                for w_sb, dst in pairs:
                    for nh in range(2):
                        ps = mix.tile([P, 512], f32, tag="qk", name="qkps")
                        for c in range(CT):
                            mm(ps[:], lhsT=w_sb[:, j, c, :],
                               rhs=x_sb[:, c, nh * 512:(nh + 1) * 512],
                               start=(c == 0), stop=(c == CT - 1))
                        nc.vector.tensor_copy(
                            dst[:, j, nh * 512:(nh + 1) * 512], ps[:])

            emit_qk(0)
            nc.vector.memset(vp[:, :, :, D:2 * D], 1.0)

            # ---- V projection: chunk-ordered waves; overlaps the pair-0/1
            # softmax windows and the input DMA tail ----
            def emit_va(ts):
                # V heads 0-7 (psa): needed from PV(0) on
                for t in ts:
                    psa = mix.tile([P, 512], f32, tag="qk", name="psa")
                    for c in range(CT):
                        mm(psa[:], lhsT=x_sb[:, c, t * P:(t + 1) * P],
                           rhs=wv_sb[:, c, 0:512],
                           start=(c == 0), stop=(c == CT - 1))
                    nc.vector.tensor_copy(
                        vp[:, t, 0:8, 0:D],
                        psa.rearrange("p (h d) -> p h d", d=D))

            def emit_vb(ts):
                # V heads 8-11 (psb): first needed by PV(4), in pair-5's
                # window — emit late to keep early pair windows light
                for t in ts:
                    psb = mix.tile([P, 512], f32, tag="qk", name="psb")
                    for c in range(CT):
                        mm(psb[:, 0:256], lhsT=x_sb[:, c, t * P:(t + 1) * P],
                           rhs=wv_sb[:, c, 512:768],
                           start=(c == 0), stop=(c == CT - 1))
                    nc.vector.tensor_copy(
                        vp[:, t, 8:12, 0:D],
                        psb[:, 0:256].rearrange("p (h d) -> p h d", d=D))

            ets = {}

            def emit_scores_mt(j, mt, nhs=(0, 1)):
                # one PSUM tile per nh holds both heads' scores side by
                # side: the h0/h64 matmuls share one WAR gate (the same
                # EXP) so the scheduler keeps them adjacent and they
                # co-run in the two PE row-tile halves.
                for nh in nhs:
                    s = ps_s.tile([P, N], f32, tag="s", name=f"s_{nh}")
                    et = etp.tile([P, N], f16, tag="et", name=f"et_{nh}")
                    ets[(j, nh, mt)] = et
                    for hh in range(2):   # adjacent => PE row-tiling
                        r0 = hh * D
                        mm(s[:, hh * 512:(hh + 1) * 512],
                           lhsT=kt[r0:r0 + D, j, mt * P:(mt + 1) * P],
                           rhs=qt[r0:r0 + D, j, nh * 512:(nh + 1) * 512],
                           start=True, stop=True)
                    nc.scalar.activation(et[:], s[:], Exp,
                                         scale=float(SCALE))

            def emit_pv_norm(j, nhs=(0, 1), hhs=(0, 1)):
                for hh in hhs:
                    h = 2 * j + hh
                    r0 = hh * D
                    pso = {nh: ps_o.tile([P, 512], f32, tag="o",
                                         name=f"o_{nh}")
                           for nh in nhs}
                    for mt in range(NT):   # dense PV burst
                        for nh in nhs:
                            mm(pso[nh][:],
                               lhsT=vp[:, mt, h],
                               rhs=ets[(j, nh, mt)][:,
                                       hh * 512:(hh + 1) * 512],
                               start=(mt == 0), stop=(mt == NT - 1))
                    for nh in nhs:
                        sums = smp.tile([D, 512], f32, tag="sums")
                        rec = smp.tile([D, 512], f32, tag="rec")
                        nc.vector.tensor_copy(sums[:], pso[nh][D:2 * D, :])
                        nc.vector.reciprocal_approx_fast(rec[:], sums[:])
                        nc.vector.tensor_mul(
                            ot[r0:r0 + D, j, nh * 512:(nh + 1) * 512],
                            pso[nh][0:D, :], rec[:])

            yre = y.rearrange("(t p) f -> t p f", p=P)
            yq = [nc.sync, nc.scalar]

            def emit_outproj(ts, pool="s"):
                # pool="s": reuse score slots; pool="mix": use the QK/V
                # filler slots (free once all projections are done) so
                # in-flight scores keep their PSUM
                for t in ts:
                    if pool == "s":
                        py = ps_s.tile([P, N], f32, tag="s", name="py")
                        pa, pb = py[:, 0:512], py[:, 512:768]
                        pf = py[:, 0:C]
                    else:
                        pya = mix.tile([P, 512], f32, tag="qk", name="pya")
                        pyb = mix.tile([P, 512], f32, tag="qk", name="pyb")
                        pa, pb = pya[:], pyb[:, 0:256]
                        pf = None
                    for c in range(CT):
                        lh = ot[:, c, t * P:(t + 1) * P]
                        mm(pa, lhsT=lh, rhs=wp_sb[:, c, 0:512],
                           start=(c == 0), stop=(c == CT - 1))
                        mm(pb, lhsT=lh, rhs=wp_sb[:, c, 512:768],
                           start=(c == 0), stop=(c == CT - 1))
                    ys = outp.tile([P, C], f32, tag="ys")
                    if pf is not None:
                        nc.vector.tensor_add(ys[:], pf, bpb_sb[:])
                    else:
                        nc.vector.tensor_add(ys[:, 0:512], pa,
                                             bpb_sb[:, 0:512])
                        nc.vector.tensor_add(ys[:, 512:768], pb,
                                             bpb_sb[:, 512:768])
                    yq[t % 2].dma_start(yre[t, :, 0:512], ys[:, 0:512])
                    yq[(t + 1) % 2].dma_start(yre[t, :, 512:768],
                                              ys[:, 512:768])

            # software-pipelined: PV/normalize of pair j-1 lands after pair
            # j's first scores; QK(j+1) / V waves fill mid-pair PE gaps.
            for j in range(HP - 1):
                for mt in range(NT):
                    emit_scores_mt(j, mt)
                    if j > 0:
                        if mt == 1:
                            emit_pv_norm(j - 1, nhs=(0,), hhs=(0,))
                        elif mt == 2:
                            emit_pv_norm(j - 1, nhs=(1,), hhs=(0,))
                        elif mt == 3:
                            emit_pv_norm(j - 1, nhs=(0,), hhs=(1,))
                        elif mt == 4:
                            emit_pv_norm(j - 1, nhs=(1,), hhs=(1,))
                    if j == 0:
                        if mt == 1:
                            emit_va(range(0, 4))
                        elif mt == 5:
                            emit_va(range(4, NT))
                    elif j == 1 and mt == 6:
                        emit_vb(range(0, 3))
                    elif j == 2 and mt == 6:
                        emit_vb(range(3, 6))
                    elif j == 3 and mt == 6:
                        emit_vb(range(6, NT))
                    if mt == 5:
                        emit_qk(j + 1, 0)
                    elif mt == 7:
                        emit_qk(j + 1, 1)
            # pair 5: all nh0 softmax first so PV(5,nh0) and the first
            # output-projection half overlap the nh1 softmax stretch
            j = HP - 1
            for mt in range(NT):
                emit_scores_mt(j, mt, nhs=(0,))
                if mt == 1:
                    emit_pv_norm(j - 1, nhs=(0,), hhs=(0,))
                elif mt == 2:
                    emit_pv_norm(j - 1, nhs=(1,), hhs=(0,))
                elif mt == 3:
                    emit_pv_norm(j - 1, nhs=(0,), hhs=(1,))
                elif mt == 4:
                    emit_pv_norm(j - 1, nhs=(1,), hhs=(1,))
            emit_pv_norm(j, (0,))
            for mt in range(NT):
                emit_scores_mt(j, mt, nhs=(1,))
                if mt == 1:
                    emit_outproj(range(0, 2), pool="mix")
                elif mt == 3:
                    emit_outproj(range(2, 4), pool="mix")
            emit_pv_norm(j, (1,))
            emit_outproj(range(4, NT))

    nc.compile()
    return nc


def _get_nc():
    if "nc" not in _cache:
        _cache["nc"] = _build_nc()
    return _cache["nc"]


def _make_in_maps(x, Wq, Wk, Wv, Wp, bp):
    x = np.asarray(x, dtype=np.float32)

    def qk_layout(w):
        # [H, D, C] -> [(hp hh d), (ct p)] -> [hp, p, ct, (hh d)]
        wT = np.asarray(w, np.float32).reshape(H * D, C).T  # [C, H*D]
        wr = wT.reshape(CT, P, HP, P).transpose(2, 1, 0, 3)  # [hp, p, ct, m]
        return np.ascontiguousarray(wr.astype(np.float16))

    wq_h = qk_layout(Wq)
    wk_h = qk_layout(Wk)
    wv_h = np.ascontiguousarray(
        np.asarray(Wv, np.float32).reshape(H * D, C).T
        .reshape(CT, P, H * D).astype(np.float16))
    wp_h = np.ascontiguousarray(
        np.asarray(Wp, np.float32).T.reshape(CT, P, C).astype(np.float16))
    bpb = np.ascontiguousarray(
        np.broadcast_to(np.asarray(bp, np.float32), (P, C)))
    in_maps = []
    for b in range(NCORES):
        xb = np.ascontiguousarray(
            x[b].T.reshape(CT, P, N).astype(np.float16))
        in_maps.append({
            "xc": xb, "wq": wq_h, "wk": wk_h, "wv": wv_h, "wp": wp_h,
            "bpb": bpb,
        })
    return in_maps


def run(x, Wq, Wk, Wv, Wp, bp, trace=False):
    from concourse.bass_utils import run_bass_kernel_spmd
    nc = _get_nc()
    in_maps = _make_in_maps(x, Wq, Wk, Wv, Wp, bp)
    res = run_bass_kernel_spmd(nc, in_maps, list(range(NCORES)), trace=trace)
    out = np.stack([res.results[b]["y"] for b in range(NCORES)])
    return out, res


def kernel(x, Wq, Wk, Wv, Wp, bp):
    out, _ = run(x, Wq, Wk, Wv, Wp, bp)
    return out


# revision 37
# speedup vs baseline: 1.0075x; 1.0075x over previous
"""Trainium2 Bass kernel for nn_Attentionv2 (B=8, N=1024, C=768, H=12, D=64).

Strategy: data-parallel over batch — one batch element per NeuronCore (8 cores).
Per core, multi-head attention is computed entirely in the "transposed"
orientation so no on-chip transposes are needed:

  QT[h*64+d, n] = sum_c WqT[c, h*64+d] * xT[c, n]     (head-pair tiles)
  KT likewise; V[n, h*64+d] = sum_c xT[c, n-tile] * WvT[c, :]
  ST[m, n]  = sum_d KT[d, m] * QT[d, n]               (scores transposed)
  ET        = exp(ST * 1/8)                            (no max-subtraction:
                                                        scores are O(1) here)
  PV lhsT   = [V_h | ones(64 cols)]  =>  out rows 0-63 = OT_h (unnorm),
               rows 64-127 = softmax denominator replicated 64x
  OT_norm   = OT * (1/Z)                               (reciprocal via DVE)
  y[n, o]   = sum_c OT_norm[c, n] * WpT[c, o] + bp[o]

All matmul operands are fp16: fp8 was tried and rejected — the attention
output is a near-uniform mean over 1024 keys, so quantization noise on
V/ET/scores passes through at full relative strength (~5% >> the 2e-2
gate).  Accumulation is fp32 in PSUM.

Schedule: host-prepped chunk-contiguous input layouts; pair-0 weights
land first and QK(0) accumulates as x arrives so the softmax pipe starts
early; dummy warm-up matmuls spin the PE from t~6us so HAM unthrottles
to 2.4GHz before real work; V-projection waves run under the pair-0/1
softmax windows; scores PSUM is triple-buffered so pair-boundary WARs
don't stall the ACT pipe; output projection interleaves with the last
pair's PV; y streams out over three DMA queues.
"""

import numpy as np

P = 128
B, N, C = 8, 1024, 768
H, D = 12, 64
SCALE = D ** -0.5  # 0.125
CT = C // P   # 6 contraction chunks
NT = N // P   # 8 sequence tiles
HP = H // 2   # 6 head pairs
NCORES = 8
NWARM = 12    # dummy PE warm-up matmuls

_cache = {}


def _build_nc():
    import concourse.bass as bass
    import concourse.mybir as mybir
    import concourse.tile as tile
    from concourse import bacc

    f32 = mybir.dt.float32
    f16 = mybir.dt.float16
    Exp = mybir.ActivationFunctionType.Exp

    nc = bacc.Bacc("TRN2", target_bir_lowering=False, debug=False,
                   enable_asserts=False)

    # host-prepped layouts (see _make_in_maps):
    #   xc  [CT, 128, N]        f16   x^T c-chunked
    #   wq  [HP, 128, CT, 128]  f16   per head-pair j: [p, c, (hh d)]
    #   wk  [HP, 128, CT, 128]  f16
    #   wv  [CT, 128, H*D]      f16
    #   wp  [CT, 128, C]        f16
    #   bpb [128, C]            f32   bias broadcast
    xc = nc.dram_tensor("xc", [CT, P, N], f16, kind="ExternalInput").ap()
    wq = nc.dram_tensor("wq", [HP, P, CT, P], f16, kind="ExternalInput").ap()
    wk = nc.dram_tensor("wk", [HP, P, CT, P], f16, kind="ExternalInput").ap()
    wv = nc.dram_tensor("wv", [CT, P, H * D], f16, kind="ExternalInput").ap()
    wp = nc.dram_tensor("wp", [CT, P, C], f16, kind="ExternalInput").ap()
    bpb = nc.dram_tensor("bpb", [P, C], f32, kind="ExternalInput").ap()
    y = nc.dram_tensor("y", [N, C], f32, kind="ExternalOutput").ap()

    mm = nc.tensor.matmul

    with tile.TileContext(nc) as tc:
        with tc.tile_pool(name="persist", bufs=1) as persist, \
             tc.tile_pool(name="mix", bufs=2, space="PSUM") as mix, \
             tc.tile_pool(name="ps_o", bufs=2, space="PSUM") as ps_o, \
             tc.tile_pool(name="et", bufs=32) as etp, \
             tc.tile_pool(name="sm", bufs=4) as smp, \
             tc.tile_pool(name="ps_s", bufs=2, space="PSUM") as ps_s, \
             tc.tile_pool(name="outp", bufs=3) as outp:
            qt = persist.tile([P, HP, N], f16)        # QT: head pair j rows
            kt = persist.tile([P, HP, N], f16)
            vp = persist.tile([P, NT, H, 2 * D], f16)  # [Vh | ones]
            ot = persist.tile([P, HP, N], f16)        # normalized OT stacked
            x_sb = persist.tile([P, CT, N], f16)
            wq_sb = persist.tile([P, HP, CT, P], f16)
            wk_sb = persist.tile([P, HP, CT, P], f16)
            wv_sb = persist.tile([P, CT, H * D], f16)
            wp_sb = persist.tile([P, CT, C], f16)
            bpb_sb = persist.tile([P, C], f32)
            warm = persist.tile([P, 512], f16)

            nc.vector.memset(warm[:], 0.125)

            # ---- input DMA: chunk-contiguous; queue FIFO acts as a
            # throttle — late-needed weights sit behind x/wv on the HWDGE
            # queues so they can't steal bandwidth from the critical path;
            # gpsimd carries only the early pair-0/1 weights ----
            # scalar queue carries ONLY the three early x chunks: its
            # sequencer must be free before the first EXP dispatches.
            # All late-needed weights go on the self-throttled gpsimd
            # queue (dedicated engine, ~1 transfer / 1.5us — plenty).
            xq = [nc.sync, nc.scalar]
            for c in range(CT):
                xq[c % 2].dma_start(x_sb[:, c, :], xc[c])
            nc.gpsimd.dma_start(wq_sb[:, 0], wq[0])
            nc.gpsimd.dma_start(wk_sb[:, 0], wk[0])
            nc.gpsimd.dma_start(wq_sb[:, 1], wq[1])
            nc.gpsimd.dma_start(wk_sb[:, 1], wk[1])
            nc.sync.dma_start(wv_sb[:, 0:3],
                              wv[0:3].rearrange("c p f -> p c f"))
            nc.gpsimd.dma_start(wv_sb[:, 3:6],
                                wv[3:6].rearrange("c p f -> p c f"))
            for j in range(2, HP):
                nc.gpsimd.dma_start(wq_sb[:, j], wq[j])
                nc.gpsimd.dma_start(wk_sb[:, j], wk[j])
            nc.sync.dma_start(wp_sb[:], wp.rearrange("c p f -> p c f"))
            nc.gpsimd.dma_start(bpb_sb[:], bpb)

            # ---- PE warm-up: dummy matmuls so HAM unthrottles to 2.4GHz
            # during the DMA window ----
            wps = ps_o.tile([P, 512], f32, tag="o", name="warmps")
            for i in range(NWARM):
                mm(wps[:], lhsT=warm[:, 0:128], rhs=warm[:],
                   start=(i == 0), stop=(i == NWARM - 1))

            # ---- QK projection for pair j ----
            def emit_qk(j, half=None):
                pairs = ((wq_sb, qt), (wk_sb, kt))
                if half is not None:
                    pairs = (pairs[half],)
                for w_sb, dst in pairs:
                    for nh in range(2):
                        ps = mix.tile([P, 512], f32, tag="qk", name="qkps")
                        for c in range(CT):
                            mm(ps[:], lhsT=w_sb[:, j, c, :],
                               rhs=x_sb[:, c, nh * 512:(nh + 1) * 512],
                               start=(c == 0), stop=(c == CT - 1))
                        nc.vector.tensor_copy(
                            dst[:, j, nh * 512:(nh + 1) * 512], ps[:])

            emit_qk(0)
            nc.vector.memset(vp[:, :, :, D:2 * D], 1.0)

            # ---- V projection: chunk-ordered waves; overlaps the pair-0/1
            # softmax windows and the input DMA tail ----
            def emit_va(ts):
                # V heads 0-7 (psa): needed from PV(0) on
                for t in ts:
                    psa = mix.tile([P, 512], f32, tag="qk", name="psa")
                    for c in range(CT):
                        mm(psa[:], lhsT=x_sb[:, c, t * P:(t + 1) * P],
                           rhs=wv_sb[:, c, 0:512],
                           start=(c == 0), stop=(c == CT - 1))
                    nc.vector.tensor_copy(
                        vp[:, t, 0:8, 0:D],
                        psa.rearrange("p (h d) -> p h d", d=D))

            def emit_vb(ts):
                # V heads 8-11 (psb): first needed by PV(4), in pair-5's
                # window — emit late to keep early pair windows light
                for t in ts:
                    psb = mix.tile([P, 512], f32, tag="qk", name="psb")
                    for c in range(CT):
                        mm(psb[:, 0:256], lhsT=x_sb[:, c, t * P:(t + 1) * P],
                           rhs=wv_sb[:, c, 512:768],
                           start=(c == 0), stop=(c == CT - 1))
                    nc.vector.tensor_copy(
                        vp[:, t, 8:12, 0:D],
                        psb[:, 0:256].rearrange("p (h d) -> p h d", d=D))

            ets = {}

            def emit_scores_mt(j, mt, nhs=(0, 1)):
                # one PSUM tile per nh holds both heads' scores side by
                # side: the h0/h64 matmuls share one WAR gate (the same
                # EXP) so the scheduler keeps them adjacent and they
                # co-run in the two PE row-tile halves.
                for nh in nhs:
                    s = ps_s.tile([P, N], f32, tag="s", name=f"s_{nh}")
                    et = etp.tile([P, N], f16, tag="et", name=f"et_{nh}")
                    ets[(j, nh, mt)] = et
                    for hh in range(2):   # adjacent => PE row-tiling
                        r0 = hh * D
                        mm(s[:, hh * 512:(hh + 1) * 512],
                           lhsT=kt[r0:r0 + D, j, mt * P:(mt + 1) * P],
                           rhs=qt[r0:r0 + D, j, nh * 512:(nh + 1) * 512],
                           start=True, stop=True)
                    nc.scalar.activation(et[:], s[:], Exp,
                                         scale=float(SCALE))

            def emit_pv_norm(j, nhs=(0, 1), hhs=(0, 1)):
                for hh in hhs:
                    h = 2 * j + hh
                    r0 = hh * D
                    pso = {nh: ps_o.tile([P, 512], f32, tag="o",
                                         name=f"o_{nh}")
                           for nh in nhs}
                    for mt in range(NT):   # dense PV burst
                        for nh in nhs:
                            mm(pso[nh][:],
                               lhsT=vp[:, mt, h],
                               rhs=ets[(j, nh, mt)][:,
                                       hh * 512:(hh + 1) * 512],
                               start=(mt == 0), stop=(mt == NT - 1))
                    for nh in nhs:
                        sums = smp.tile([D, 512], f32, tag="sums")
                        rec = smp.tile([D, 512], f32, tag="rec")
                        nc.vector.tensor_copy(sums[:], pso[nh][D:2 * D, :])
                        nc.vector.reciprocal_approx_fast(rec[:], sums[:])
                        nc.vector.tensor_mul(
                            ot[r0:r0 + D, j, nh * 512:(nh + 1) * 512],
                            pso[nh][0:D, :], rec[:])

            yre = y.rearrange("(t p) f -> t p f", p=P)
            yq = [nc.sync, nc.scalar]

            def emit_outproj(ts, pool="s"):
                # pool="s": reuse score slots; pool="mix": use the QK/V
                # filler slots (free once all projections are done) so
                # in-flight scores keep their PSUM
                for t in ts:
                    if pool == "s":
                        py = ps_s.tile([P, N], f32, tag="s", name="py")
                        pa, pb = py[:, 0:512], py[:, 512:768]
                        pf = py[:, 0:C]
                    else:
                        pya = mix.tile([P, 512], f32, tag="qk", name="pya")
                        pyb = mix.tile([P, 512], f32, tag="qk", name="pyb")
                        pa, pb = pya[:], pyb[:, 0:256]
                        pf = None
                    for c in range(CT):
                        lh = ot[:, c, t * P:(t + 1) * P]
                        mm(pa, lhsT=lh, rhs=wp_sb[:, c, 0:512],
                           start=(c == 0), stop=(c == CT - 1))
                        mm(pb, lhsT=lh, rhs=wp_sb[:, c, 512:768],
                           start=(c == 0), stop=(c == CT - 1))
                    ys = outp.tile([P, C], f32, tag="ys")
                    if pf is not None:
                        nc.vector.tensor_add(ys[:], pf, bpb_sb[:])
                    else:
                        nc.vector.tensor_add(ys[:, 0:512], pa,
                                             bpb_sb[:, 0:512])
                        nc.vector.tensor_add(ys[:, 512:768], pb,
                                             bpb_sb[:, 512:768])
                    yq[t % 2].dma_start(yre[t, :, 0:512], ys[:, 0:512])
                    yq[(t + 1) % 2].dma_start(yre[t, :, 512:768],
                                              ys[:, 512:768])

            # software-pipelined: PV/normalize of pair j-1 lands after pair
            # j's first scores; QK(j+1) / V waves fill mid-pair PE gaps.
            for j in range(HP - 1):
                for mt in range(NT):
                    emit_scores_mt(j, mt)
                    if j > 0:
                        if mt == 1:
                            emit_pv_norm(j - 1, nhs=(0,), hhs=(0,))
                        elif mt == 2:
                            emit_pv_norm(j - 1, nhs=(1,), hhs=(0,))
                        elif mt == 3:
                            emit_pv_norm(j - 1, nhs=(0,), hhs=(1,))
                        elif mt == 4:
                            emit_pv_norm(j - 1, nhs=(1,), hhs=(1,))
                    if j == 0:
                        if mt == 1:
                            emit_va(range(0, 4))
                        elif mt == 5:
                            emit_va(range(4, NT))
                    elif j == 1 and mt == 6:
                        emit_vb(range(0, 3))
                    elif j == 2 and mt == 6:
                        emit_vb(range(3, 6))
                    elif j == 3 and mt == 6:
                        emit_vb(range(6, NT))
                    if mt == 5:
                        emit_qk(j + 1, 0)
                    elif mt == 7:
                        emit_qk(j + 1, 1)
            # pair 5: all nh0 softmax first so PV(5,nh0) and the first
            # output-projection half overlap the nh1 softmax stretch
            j = HP - 1
            for mt in range(NT):
                emit_scores_mt(j, mt, nhs=(0,))
                if mt == 1:
                    emit_pv_norm(j - 1, nhs=(0,), hhs=(0,))
                elif mt == 2:
                    emit_pv_norm(j - 1, nhs=(1,), hhs=(0,))
                elif mt == 3:
                    emit_pv_norm(j - 1, nhs=(0,), hhs=(1,))
                elif mt == 4:
                    emit_pv_norm(j - 1, nhs=(1,), hhs=(1,))
            emit_pv_norm(j, (0,))
            for mt in range(NT):
                emit_scores_mt(j, mt, nhs=(1,))
                if mt == 1:
                    emit_outproj(range(0, 2), pool="mix")
                elif mt == 3:
                    emit_outproj(range(2, 4), pool="mix")
            emit_pv_norm(j, (1,))
            emit_outproj(range(4, NT))
            # anti-DCE consumer for the warm-up PSUM, lowest priority so
            # it never delays the prologue casts on DVE
            nc.vector.tensor_copy(warm[:], wps[:])

    nc.compile()
    return nc


def _get_nc():
    if "nc" not in _cache:
        _cache["nc"] = _build_nc()
    return _cache["nc"]


def _make_in_maps(x, Wq, Wk, Wv, Wp, bp):
    x = np.asarray(x, dtype=np.float32)

    def qk_layout(w):
        # [H, D, C] -> [(hp hh d), (ct p)] -> [hp, p, ct, (hh d)]
        wT = np.asarray(w, np.float32).reshape(H * D, C).T  # [C, H*D]
        wr = wT.reshape(CT, P, HP, P).transpose(2, 1, 0, 3)  # [hp, p, ct, m]
        return np.ascontiguousarray(wr.astype(np.float16))

    wq_h = qk_layout(Wq)
    wk_h = qk_layout(Wk)
    wv_h = np.ascontiguousarray(
        np.asarray(Wv, np.float32).reshape(H * D, C).T
        .reshape(CT, P, H * D).astype(np.float16))
    wp_h = np.ascontiguousarray(
        np.asarray(Wp, np.float32).T.reshape(CT, P, C).astype(np.float16))
    bpb = np.ascontiguousarray(
        np.broadcast_to(np.asarray(bp, np.float32), (P, C)))
    in_maps = []
    for b in range(NCORES):
        xb = np.ascontiguousarray(
            x[b].T.reshape(CT, P, N).astype(np.float16))
        in_maps.append({
            "xc": xb, "wq": wq_h, "wk": wk_h, "wv": wv_h, "wp": wp_h,
            "bpb": bpb,
        })
    return in_maps


def run(x, Wq, Wk, Wv, Wp, bp, trace=False):
    from concourse.bass_utils import run_bass_kernel_spmd
    nc = _get_nc()
    in_maps = _make_in_maps(x, Wq, Wk, Wv, Wp, bp)
    res = run_bass_kernel_spmd(nc, in_maps, list(range(NCORES)), trace=trace)
    out = np.stack([res.results[b]["y"] for b in range(NCORES)])
    return out, res


def kernel(x, Wq, Wk, Wv, Wp, bp):
    out, _ = run(x, Wq, Wk, Wv, Wp, bp)
    return out
